# revision 8
# baseline (speedup 1.0000x reference)
"""
CastratedGAT Trainium2 kernel (8 NeuronCores, SPMD, full-I/O contract).

Algorithm
---------
Reference computes a single GATConv-like layer:
  h = (x @ W).reshape(N, H, C);  a_src = sum(h*att_src, -1);  a_dst likewise
  per edge (dst <- src):  alpha = leaky_relu(a_src[src] + a_dst[dst], 0.2)
  segment softmax over each dst's neighborhood (incl. self loop), dropout on p,
  out[dst] = sum p * h[src]  (+ self term), + bias.

Key identity used on device: with ex = exp(alpha) (no max-subtraction needed --
alpha is O(1) so exp never overflows), and denom = segment_sum(ex),
  out[d,h,:] = ( sum_e ex*dp*h[src] ) / denom[d,h]
so the dropout mask only scales the numerator and the denominator never has to
be scattered back to edges.

Sharding: nodes are range-partitioned across the 8 cores (6250 each). Edges are
bucketed by destination (host-side sort), so segment reduction and the output
write stay core-local. Every core computes the full node table
T1[n] = [h(256) | a_src(8) | a_dst(8)] itself (replicated matmul; avoids
collectives) and then gathers rows of T1 for its local edges.

Edge phase: edges (incl. self loops) sorted by dst are packed into
"super-chunks" of <=1024 edge slots covering <=64 consecutive dst nodes.
Per super-chunk:
  - one batched indirect DMA gathers 1024 rows x 264 elems of T1 (h|a_src)
  - one batched indirect DMA gathers 1024 x 8 a_dst values (element_offset=264)
  - per 128-edge chunk, a 0/1 selection matrix S[e, d] = (dst_local[e]==d) is
    built with one iota-compare; then matmul psum[64,264] += S.T @ [ex*dp*h | ex]
    performs the segment scatter-add for numerator AND denominator at once
  - epilogue divides by denom, adds bias, and scatters 64 rows to the output
    shard via indirect DMA with OOB indices (skipped) marking padding rows.
"""

import math

import numpy as np

# problem constants (hardcoded per contract -- kernel.py is self-contained)
N = 50000
E = 800000
F_IN = 128
H = 8
C = 32
HC = H * C  # 256
NCORES = 8
NLOC = N // NCORES  # 6250

P = 128           # partitions / edges per chunk
SC_E = 1024       # edge slots per super-chunk (8 chunks)
SC_K = SC_E // P  # 8 chunks per super-chunk
SC_D = 64         # max distinct dsts per super-chunk
GRP = 512         # phase-0 node-group per x DMA
TROW = 272        # T1 row: h(256) + a_src(8) + a_dst(8)
OOB = 1 << 20

LAST_EXEC_NS = None
LAST_RESULTS = None


# ---------------------------------------------------------------- host prep

def _prep_params(n, e, ncores):
    nloc = n // ncores
    npad = int(math.ceil(n / GRP)) * GRP
    return nloc, npad


def _pack_core(dst, src, dp, base, nloc, nsc_cap=None):
    """Pack one core's (dst-sorted) edges into super-chunk arrays.

    dst/src: int32 [e_core] with dst in [base, base+nloc), sorted by dst.
    dp: float32 [e_core, H].
    Returns meta [nsc,128,16] i32, fv [nsc,128,72] f32, oidx [64, nsc] i32.
    """
    e_core = dst.shape[0]
    dloc_all = dst - base  # 0..nloc-1, sorted
    cnt = np.bincount(dloc_all, minlength=nloc)
    assert cnt.min() >= 1  # self loops guarantee this
    assert cnt.max() <= SC_E

    # greedy cut into super-chunks: <= SC_E edges, <= SC_D distinct dsts
    cuts = [0]  # dst-id boundaries
    acc_e = 0
    d0 = 0
    for d in range(nloc):
        c = int(cnt[d])
        if acc_e + c > SC_E or d - d0 >= SC_D:
            cuts.append(d)
            d0 = d
            acc_e = c
        else:
            acc_e += c
    cuts.append(nloc)
    nsc = len(cuts) - 1

    row_start = np.zeros(nloc + 1, dtype=np.int64)
    np.cumsum(cnt, out=row_start[1:])

    import ml_dtypes
    meta = np.zeros((nsc, P, 16), dtype=np.int32)
    fv = np.full((nsc, P, 8), 255.0, dtype=np.float32)  # dloc padding -> S row 0
    dpm = np.zeros((nsc, P, 64), dtype=ml_dtypes.bfloat16)
    oidx = np.full((SC_D, nsc), OOB, dtype=np.int32)

    # per-edge sc id and position inside sc
    sc_first_edge = row_start[np.asarray(cuts[:-1])]
    sc_of_edge = np.searchsorted(sc_first_edge, np.arange(e_core), side="right") - 1
    pos = np.arange(e_core) - sc_first_edge[sc_of_edge]
    assert pos.max() < SC_E
    k = (pos // P).astype(np.int64)
    p = (pos % P).astype(np.int64)
    w0 = np.asarray(cuts[:-1], dtype=np.int64)  # local dst window starts

    meta[sc_of_edge, p, k] = src
    meta[sc_of_edge, p, 8 + k] = dst
    fv[sc_of_edge, p, k] = (dloc_all - w0[sc_of_edge]).astype(np.float32)
    dpm[sc_of_edge[:, None], p[:, None], k[:, None] * 8 + np.arange(8)[None, :]] = \
        dp.astype(ml_dtypes.bfloat16)

    for s in range(nsc):
        nd = cuts[s + 1] - cuts[s]
        oidx[:nd, s] = np.arange(cuts[s], cuts[s + 1], dtype=np.int32)

    if nsc_cap is not None:
        assert nsc <= nsc_cap
        if nsc < nsc_cap:
            pad = nsc_cap - nsc
            meta = np.concatenate([meta, np.zeros((pad, P, 16), np.int32)], axis=0)
            fv = np.concatenate(
                [fv, np.full((pad, P, 8), 255.0, np.float32)], axis=0)
            dpm = np.concatenate(
                [dpm, np.zeros((pad, P, 64), ml_dtypes.bfloat16)], axis=0)
            oidx = np.concatenate(
                [oidx, np.full((SC_D, pad), OOB, np.int32)], axis=1)
    return meta, fv, dpm, oidx, nsc


def _host_prep(x, edge_index, dp_mask, dp_mask_self, W, att_src, att_dst, bias,
               n, e, ncores):
    nloc, npad = _prep_params(n, e, ncores)
    h = H

    dst = np.asarray(edge_index[0], dtype=np.int64)
    src = np.asarray(edge_index[1], dtype=np.int64)
    loops = np.arange(n, dtype=np.int64)
    all_dst = np.concatenate([dst, loops])
    all_src = np.concatenate([src, loops])
    all_dp = np.concatenate([np.asarray(dp_mask, np.float32),
                             np.asarray(dp_mask_self, np.float32)], axis=0)

    order = np.argsort(all_dst, kind="stable")
    all_dst = all_dst[order]
    all_src = all_src[order]
    all_dp = all_dp[order]

    # per-core slices (dst ranges are contiguous after the sort)
    core_lo = np.searchsorted(all_dst, np.arange(ncores) * nloc)
    core_hi = np.searchsorted(all_dst, (np.arange(ncores) + 1) * nloc)

    packs = []
    for m in range(ncores):
        lo, hi = core_lo[m], core_hi[m]
        packs.append(_pack_core(all_dst[lo:hi].astype(np.int32),
                                all_src[lo:hi].astype(np.int32),
                                all_dp[lo:hi], m * nloc, nloc))
    nsc = max(pk[4] for pk in packs)
    packs = [
        _pack_core(all_dst[core_lo[m]:core_hi[m]].astype(np.int32),
                   all_src[core_lo[m]:core_hi[m]].astype(np.int32),
                   all_dp[core_lo[m]:core_hi[m]], m * nloc, nloc, nsc_cap=nsc)
        for m in range(ncores)
    ]

    # shared (replicated) tensors (bf16 for TensorEngine operands)
    import ml_dtypes
    bf16 = ml_dtypes.bfloat16
    xT = np.zeros((F_IN, npad), dtype=bf16)
    xT[:, :n] = np.asarray(x, np.float32).T.astype(bf16)
    Wf = np.asarray(W, np.float32)                      # [128, 256]
    A = np.zeros((HC, 2 * h), dtype=np.float32)
    for hd in range(h):
        A[hd * C:(hd + 1) * C, hd] = np.asarray(att_src, np.float32)[hd]
        A[hd * C:(hd + 1) * C, h + hd] = np.asarray(att_dst, np.float32)[hd]
    WA = (Wf @ A).astype(bf16)                           # [128, 16]
    Wb = Wf.astype(bf16)
    biasr = np.broadcast_to(np.asarray(bias, np.float32)[None, :],
                            (P, HC)).copy()

    in_maps = []
    for m in range(ncores):
        meta, fv, dpm, oidx, _ = packs[m]
        in_maps.append({
            "xT": xT, "W": Wb, "WA": WA, "biasr": biasr,
            "meta": meta, "fv": fv, "dpm": dpm, "oidx": oidx,
        })
    return in_maps, nsc, nloc, npad


# ---------------------------------------------------------------- device side

def _build(nsc, nloc, npad):
    import concourse.bass as bass
    import concourse.bacc as bacc
    import concourse.mybir as mybir
    from concourse.tile import TileContext

    f32 = mybir.dt.float32
    i32 = mybir.dt.int32
    bf16 = mybir.dt.bfloat16
    DT = bf16

    nc = bacc.Bacc(None, target_bir_lowering=False)
    xT = nc.dram_tensor("xT", [F_IN, npad], bf16, kind="ExternalInput")
    W = nc.dram_tensor("W", [F_IN, HC], bf16, kind="ExternalInput")
    WA = nc.dram_tensor("WA", [F_IN, 2 * H], bf16, kind="ExternalInput")
    biasr = nc.dram_tensor("biasr", [P, HC], f32, kind="ExternalInput")
    meta = nc.dram_tensor("meta", [nsc, P, 16], i32, kind="ExternalInput")
    fv = nc.dram_tensor("fv", [nsc, P, 8], f32, kind="ExternalInput")
    dpm = nc.dram_tensor("dpm", [nsc, P, 64], bf16, kind="ExternalInput")
    oidx = nc.dram_tensor("oidx", [SC_D, nsc], i32, kind="ExternalInput")
    out = nc.dram_tensor("out", [nloc, HC], f32, kind="ExternalOutput")
    T1 = nc.dram_tensor("T1", [npad, TROW], DT, kind="Internal")

    ngrp = npad // GRP

    with TileContext(nc) as tc:
        with (
            tc.tile_pool(name="const", bufs=1) as cpool,
            tc.tile_pool(name="xt", bufs=3) as xpool,
            tc.tile_pool(name="t1o", bufs=4) as t1pool,
            tc.tile_pool(name="ps0", bufs=4, space="PSUM") as ps0,
            tc.tile_pool(name="stream", bufs=3) as spool,
            tc.tile_pool(name="gath", bufs=6) as gpool,
            tc.tile_pool(name="work", bufs=3) as wpool,
            tc.tile_pool(name="ps1", bufs=2, space="PSUM") as ps1,
        ):
            # constants
            w_sb = cpool.tile([F_IN, HC], bf16)
            nc.sync.dma_start(out=w_sb[:], in_=W[:, :])
            wa_sb = cpool.tile([F_IN, 2 * H], bf16)
            nc.sync.dma_start(out=wa_sb[:], in_=WA[:, :])
            bias_sb = cpool.tile([P, HC], f32)
            nc.sync.dma_start(out=bias_sb[:], in_=biasr[:, :])
            oidx_sb = cpool.tile([SC_D, nsc], i32)
            nc.sync.dma_start(out=oidx_sb[:], in_=oidx[:, :])
            iota_i = cpool.tile([P, SC_D], i32)
            nc.gpsimd.iota(iota_i[:], pattern=[[1, SC_D]], base=0,
                           channel_multiplier=0)
            iota_f = cpool.tile([P, SC_D], f32)
            nc.vector.tensor_copy(out=iota_f[:], in_=iota_i[:])
            bc_reg = nc.gpsimd.to_reg(nloc - 1)

            # ---------------- phase 0: T1[n] = [x@W | x@WA] ----------------
            for g in range(ngrp):
                xt = xpool.tile([F_IN, GRP], bf16, tag="xt")
                nc.sync.dma_start(out=xt[:], in_=xT[:, g * GRP:(g + 1) * GRP])
                for s in range(GRP // P):
                    psum = ps0.tile([P, TROW], f32, tag="p0")
                    lhsT = xt[:, s * P:(s + 1) * P]
                    nc.tensor.matmul(psum[:, 0:HC], lhsT, w_sb[:],
                                     start=True, stop=True)
                    nc.tensor.matmul(psum[:, HC:TROW], lhsT, wa_sb[:],
                                     start=True, stop=True)
                    t1t = t1pool.tile([P, TROW], DT, tag="t1t")
                    nc.vector.tensor_copy(out=t1t[:], in_=psum[:])
                    r0 = (g * (GRP // P) + s) * P
                    nc.sync.dma_start(out=T1[r0:r0 + P, :], in_=t1t[:])

            # ---------------- phase 1: edge aggregation --------------------
            for sc in range(nsc):
                meta_t = spool.tile([P, 16], i32, tag="meta")
                nc.sync.dma_start(out=meta_t[:], in_=meta[sc, :, :])
                fv_t = spool.tile([P, 8], f32, tag="fv")
                nc.sync.dma_start(out=fv_t[:], in_=fv[sc, :, :])
                dp_t = spool.tile([P, 64], bf16, tag="dp")
                nc.sync.dma_start(out=dp_t[:], in_=dpm[sc, :, :])

                psum = ps1.tile([SC_D, 264], f32, tag="p1")
                for k in range(SC_K):
                    G = gpool.tile([P, 264], DT, tag="G")
                    nc.gpsimd.indirect_dma_start(
                        out=G[:], out_offset=None, in_=T1[:, :],
                        in_offset=bass.IndirectOffsetOnAxis(
                            ap=meta_t[:, k:k + 1], axis=0),
                    )
                    AD = gpool.tile([P, H], DT, tag="AD")
                    nc.gpsimd.indirect_dma_start(
                        out=AD[:], out_offset=None, in_=T1[:, :],
                        in_offset=bass.IndirectOffsetOnAxis(
                            ap=meta_t[:, 8 + k:9 + k], axis=0),
                        element_offset=264,
                    )
                    S = wpool.tile([P, SC_D], DT, tag="S")
                    nc.vector.tensor_tensor(
                        out=S[:], in0=iota_f[:],
                        in1=fv_t[:, k:k + 1].to_broadcast([P, SC_D]),
                        op=mybir.AluOpType.is_equal)
                    alpha = wpool.tile([P, H], f32, tag="alpha")
                    nc.vector.tensor_tensor(
                        out=alpha[:], in0=G[:, 256:264],
                        in1=AD[:, 0:H], op=mybir.AluOpType.add)
                    lr = wpool.tile([P, H], f32, tag="lr")
                    nc.vector.scalar_tensor_tensor(
                        out=lr[:], in0=alpha[:], scalar=0.2, in1=alpha[:],
                        op0=mybir.AluOpType.mult, op1=mybir.AluOpType.max)
                    rhs = wpool.tile([P, 264], DT, tag="rhs")
                    nc.scalar.activation(out=rhs[:, 256:264], in_=lr[:],
                                         func=mybir.ActivationFunctionType.Exp)
                    dpex = wpool.tile([P, H], DT, tag="dpex")
                    nc.vector.tensor_tensor(
                        out=dpex[:], in0=rhs[:, 256:264],
                        in1=dp_t[:, k * 8:(k + 1) * 8],
                        op=mybir.AluOpType.mult)
                    nc.vector.tensor_tensor(
                        out=rhs[:, 0:256], in0=G[:, 0:256],
                        in1=dpex[:].to_broadcast([P, H, C]),
                        op=mybir.AluOpType.mult)
                    nc.tensor.matmul(psum[:], S[:], rhs[:],
                                     start=(k == 0), stop=(k == SC_K - 1))

                rec = wpool.tile([SC_D, H], f32, tag="rec")
                nc.vector.reciprocal(out=rec[:], in_=psum[:, 256:264])
                outt = wpool.tile([SC_D, HC], f32, tag="outt")
                nc.vector.tensor_tensor(
                    out=outt[:], in0=psum[:, 0:256],
                    in1=rec[:].to_broadcast([SC_D, H, C]),
                    op=mybir.AluOpType.mult)
                nc.vector.tensor_tensor(
                    out=outt[:], in0=outt[:], in1=bias_sb[0:SC_D, :],
                    op=mybir.AluOpType.add)
                nc.gpsimd.indirect_dma_start(
                    out=out[:, :],
                    out_offset=bass.IndirectOffsetOnAxis(
                        ap=oidx_sb[:, sc:sc + 1], axis=0),
                    in_=outt[:], in_offset=None,
                    bounds_check=bc_reg, oob_is_err=False,
                )
    nc.finalize()
    return nc


# ---------------------------------------------------------------- entry point

def kernel(**inputs):
    global LAST_EXEC_NS, LAST_RESULTS
    import os
    from concourse.bass_utils import run_bass_kernel_spmd

    in_maps, nsc, nloc, npad = _host_prep(
        inputs["x"], inputs["edge_index"], inputs["dp_mask"],
        inputs["dp_mask_self"], inputs["W"], inputs["att_src"],
        inputs["att_dst"], inputs["bias"], N, E, NCORES)

    nc = _build(nsc, nloc, npad)
    trace = bool(int(os.environ.get("GAT_TRACE", "0")))
    res = run_bass_kernel_spmd(nc, in_maps, core_ids=list(range(NCORES)),
                               trace=trace)
    LAST_EXEC_NS = res.exec_time_ns
    LAST_RESULTS = res
    out = np.concatenate([res.results[m]["out"] for m in range(NCORES)], axis=0)
    return out.astype(np.float32)
